# revision 9
# baseline (speedup 1.0000x reference)
"""Trainium2 Bass kernel for nn_GaussianMLPBasisLayer.

Computes, for x [B=16, N=2048, D=512] flattened to xf [Nt=32768, D]:
  att = softmax(softplus(xf @ W1 + b1) @ W2 + b2)          [Nt, K=64]
  s1 = att.T @ xf ; s2 = att.T @ xf^2 ; sa = sum(att, 0)   (global stats)
  mean = s1/sa ; var = s2/sa - mean^2
  gmm_log[n,k] = -0.5*( (x2 @ ivar.T) - 2 x @ (mean*ivar).T + q0[k] )
                 - 0.5*sum(log var)[k] - D/2*ln(2pi)
Returns (gmm_log [Nt, K], var [K, D]).

Sharding: data-parallel over Nt across 8 NeuronCores (4096 rows each);
per-core partial stats are AllReduce'd on-chip, then each core computes
its rows of the log-likelihood.

Host-side prep: x is passed both in natural layout and pre-transposed
(xT), since the D-contraction matmuls need D on partitions.
"""

import math
import numpy as np

import concourse.bass as bass
import concourse.bacc as bacc
import concourse.mybir as mybir
import concourse.tile as tile
from concourse.bass_utils import run_bass_kernel_spmd

F32 = mybir.dt.float32
F32R = mybir.dt.float32r
AF = mybir.ActivationFunctionType
ts = bass.ts

D = 512      # feature dim
H = 256      # hidden
K = 64       # components
NCORES = 8
LN2PI = float(np.log(2.0 * np.pi))


def _mm(dt):
    """bitcast helper for matmul operand dtype."""
    def f(ap):
        return ap.bitcast(dt) if dt is not F32 else ap
    return f


def build_gmm_kernel(R=4096, mmdt=F32R, n_cores=NCORES):
    """Build the SPMD kernel for R rows per core. R % 512 == 0."""
    assert R % 512 == 0
    nblk = R // 512           # 512-row blocks
    DC = D // 128             # 4 d-chunks
    HC = H // 128             # 2 h-chunks
    c = lambda ap: ap

    nc = bacc.Bacc("TRN2", target_bir_lowering=False, debug=False,
                   enable_asserts=False, num_devices=n_cores)

    # ---- DRAM I/O ----
    MD = mmdt
    x_d = nc.dram_tensor("x_sh", [R, D], MD, kind="ExternalInput").ap()
    xT_d = nc.dram_tensor("xT_sh", [D, R], MD, kind="ExternalInput").ap()
    W1_d = nc.dram_tensor("w1", [D, H], MD, kind="ExternalInput").ap()
    b1_d = nc.dram_tensor("b1", [H], F32, kind="ExternalInput").ap()
    W2_d = nc.dram_tensor("w2", [H, K], MD, kind="ExternalInput").ap()
    b2_d = nc.dram_tensor("b2r", [1, K], MD, kind="ExternalInput").ap()
    ones_d = nc.dram_tensor("ones", [128, 128], MD, kind="ExternalInput").ap()
    id_d = nc.dram_tensor("ident", [128, 128], F32, kind="ExternalInput").ap()

    gmm_d = nc.dram_tensor("gmm", [R, K], F32, kind="ExternalOutput").ap()
    var_d = nc.dram_tensor("var", [K, D], F32, kind="ExternalOutput").ap()

    from contextlib import ExitStack
    with tile.TileContext(nc) as tc, ExitStack() as ctx:
        # ---------------- pools ----------------
        ep = ctx.enter_context
        const_p = ep(tc.tile_pool(name="const", bufs=1))
        xt_p = ep(tc.tile_pool(name="xt", bufs=1))
        x_p = ep(tc.tile_pool(name="x", bufs=3))
        x2_p = ep(tc.tile_pool(name="x2", bufs=3))
        h_p = ep(tc.tile_pool(name="h", bufs=3))
        eh_p = ep(tc.tile_pool(name="eh", bufs=3))
        sm_p = ep(tc.tile_pool(name="sm", bufs=2))
        att_p = ep(tc.tile_pool(name="att", bufs=2))
        mid_p = ep(tc.tile_pool(name="mid", bufs=1))
        og_p = ep(tc.tile_pool(name="og", bufs=2))
        x2t_p = ep(tc.tile_pool(name="x2t", bufs=2))

        hps_p = ep(tc.tile_pool(name="hps", bufs=2, space="PSUM"))
        lg_p = ep(tc.tile_pool(name="lg", bufs=2, space="PSUM"))
        st_stack = ExitStack()
        st_p = st_stack.enter_context(tc.tile_pool(name="st", bufs=1,
                                                   space="PSUM"))
        dram_p = ep(tc.tile_pool(name="dram", bufs=1, space="DRAM"))

        # ---------------- constants / weights ----------------
        W1s = const_p.tile([128, DC, H], MD, tag="w1")
        nc.sync.dma_start(W1s[:], W1_d.rearrange("(c p) h -> p c h", p=128))
        W2s = const_p.tile([128, HC, K], MD, tag="w2")
        nc.sync.dma_start(W2s[:], W2_d.rearrange("(c p) k -> p c k", p=128))
        b1s = const_p.tile([128, HC], F32, tag="b1")
        nc.sync.dma_start(b1s[:], b1_d.rearrange("(c p) -> p c", p=128))
        b2s = const_p.tile([1, K], MD, tag="b2")
        nc.sync.dma_start(b2s[:], b2_d[:1, :])
        ones_s = const_p.tile([128, 128], MD, tag="ones")
        nc.sync.dma_start(ones_s[:], ones_d[:, :])
        id_s = const_p.tile([128, 128], F32, tag="id")
        nc.sync.dma_start(id_s[:], id_d[:, :])

        # persistent xT [128, DC, R]
        xTs = xt_p.tile([128, DC, R], MD, tag="xts")
        for b in range(nblk):
            nc.sync.dma_start(
                xTs[:, :, ts(b, 512)],
                xT_d.rearrange("(c p) n -> p c n", p=128)[:, :, ts(b, 512)])

        # stats accumulators (one PSUM bank each)
        s1ps = st_p.tile([K, D], F32, tag="s1")
        s2ps = st_p.tile([K, D], F32, tag="s2")
        saps = st_p.tile([K, 2], F32, tag="sa")

        # ---------------- pass 1 ----------------
        for b in range(nblk):
            # mm1: hT chunks [128h, 512n] + softplus = ln(1+exp(.+b1))
            h_sb = []
            for hc in range(HC):
                hps = hps_p.tile([128, 512], F32, tag="hps")
                for dc in range(DC):
                    nc.tensor.matmul(
                        hps[:], c(W1s[:, dc, ts(hc, 128)]),
                        c(xTs[:, dc, ts(b, 512)]),
                        start=(dc == 0), stop=(dc == DC - 1))
                eh = eh_p.tile([128, 512], F32, tag="eh")
                nc.scalar.activation(eh[:], hps[:], AF.Exp,
                                     bias=b1s[:, ts(hc, 1)])
                hsb = h_p.tile([128, 512], MD, tag="h")
                nc.scalar.activation(hsb[:], eh[:], AF.Ln, bias=1.0)
                h_sb.append(hsb)

            # mm2: logits [128, 4, 64] (one PSUM bank) + b2
            lg = lg_p.tile([128, 4, K], F32, tag="lg")
            for t in range(4):
                for hc in range(HC):
                    nc.tensor.matmul(
                        lg[:, t, :], c(h_sb[hc][:, ts(t, 128)]),
                        c(W2s[:, hc, :]), start=(hc == 0), stop=False)
                nc.tensor.matmul(lg[:, t, :], c(ones_s[:1, :128]),
                                 c(b2s[:1, :]), start=False, stop=True)

            # softmax over K (no max-sub: logits are O(+-6))
            eb = sm_p.tile([128, 4, K], F32, tag="eb")
            nc.scalar.activation(eb[:], lg[:], AF.Exp)
            sums = sm_p.tile([128, 4], F32, tag="sums")
            nc.vector.reduce_sum(sums[:], eb[:], axis=mybir.AxisListType.X)
            rec = sm_p.tile([128, 4], F32, tag="rec")
            nc.vector.reciprocal(rec[:], sums[:])
            attb = att_p.tile([128, 4, K], MD, tag="attb")
            for t in range(4):
                nc.vector.tensor_scalar_mul(attb[:, t, :], eb[:, t, :],
                                            rec[:, ts(t, 1)])

            # stats
            for t in range(4):
                i = b * 4 + t
                xt = x_p.tile([128, D], MD, tag="xt")
                nc.sync.dma_start(xt[:], x_d[ts(i, 128), :])
                x2t = x2_p.tile([128, D], MD, tag="x2t")
                nc.vector.tensor_mul(x2t[:], xt[:], xt[:])
                first = (i == 0)
                last = (i == nblk * 4 - 1)
                nc.tensor.matmul(s1ps[:], c(attb[:, t, :]), c(xt[:]),
                                 start=first, stop=last)
                nc.tensor.matmul(s2ps[:], c(attb[:, t, :]), c(x2t[:]),
                                 start=first, stop=last)
                nc.tensor.matmul(saps[:], c(attb[:, t, :]),
                                 c(ones_s[:, :2]), start=first, stop=last)

        # ---------------- mid: AllReduce + coefficients ----------------
        s1sb = mid_p.tile([K, D], F32, tag="s1sb")
        nc.scalar.activation(s1sb[:], s1ps[:], AF.Copy)
        s2sb = mid_p.tile([K, D], F32, tag="s2sb")
        nc.scalar.activation(s2sb[:], s2ps[:], AF.Copy)
        sasb = mid_p.tile([K, 1], F32, tag="sasb")
        nc.vector.tensor_copy(sasb[:], saps[:, :1])
        st_stack.close()
        gm_p = ep(tc.tile_pool(name="gm", bufs=4, space="PSUM"))

        ar_in = dram_p.tile([K, 2 * D + 1], F32, tag="arin")
        ar_out = dram_p.tile([K, 2 * D + 1], F32, tag="arout")
        nc.sync.dma_start(ar_in[:, 0:D], s1sb[:])
        nc.sync.dma_start(ar_in[:, D:2 * D], s2sb[:])
        nc.sync.dma_start(ar_in[:, 2 * D:], sasb[:])
        nc.gpsimd.collective_compute(
            "AllReduce", mybir.AluOpType.add,
            replica_groups=[list(range(n_cores))],
            ins=[ar_in.opt()], outs=[ar_out.opt()])
        red = mid_p.tile([K, 2 * D + 1], F32, tag="red")
        nc.sync.dma_start(red[:], ar_out[:])

        S1 = red[:, 0:D]
        S2 = red[:, D:2 * D]
        SA = red[:, 2 * D:]

        recs = mid_p.tile([K, 1], F32, tag="recs")
        nc.vector.reciprocal(recs[:], SA)
        mean = mid_p.tile([K, D], F32, tag="mean")
        nc.vector.tensor_scalar_mul(mean[:], S1, recs[:])
        msq = mid_p.tile([K, D], F32, tag="msq")
        nc.scalar.activation(msq[:], mean[:], AF.Square)
        vt = mid_p.tile([K, D], F32, tag="vt")
        nc.vector.tensor_scalar_mul(vt[:], S2, recs[:])
        var = mid_p.tile([K, D], F32, tag="var")
        nc.vector.tensor_sub(var[:], vt[:], msq[:])
        nc.sync.dma_start(var_d[:, :], var[:])

        iv = mid_p.tile([K, D], F32, tag="iv")
        nc.vector.reciprocal(iv[:], var[:])
        B0 = mid_p.tile([K, D], F32, tag="B0")
        nc.vector.tensor_mul(B0[:], mean[:], iv[:])
        A0 = mid_p.tile([K, D], F32, tag="A0")
        nc.vector.tensor_scalar_mul(A0[:], iv[:], -0.5)
        lv = mid_p.tile([K, D], F32, tag="lv")
        nc.scalar.activation(lv[:], var[:], AF.Ln)

        q1 = mid_p.tile([K, D], F32, tag="q1")
        nc.vector.tensor_mul(q1[:], B0[:], mean[:])
        q0 = mid_p.tile([K, 1], F32, tag="q0")
        nc.vector.reduce_sum(q0[:], q1[:], axis=mybir.AxisListType.X)
        slv = mid_p.tile([K, 1], F32, tag="slv")
        nc.vector.reduce_sum(slv[:], lv[:], axis=mybir.AxisListType.X)
        tsum = mid_p.tile([K, 1], F32, tag="tsum")
        nc.vector.tensor_add(tsum[:], q0[:], slv[:])
        csb = mid_p.tile([K, 1], F32, tag="csb")
        nc.vector.tensor_scalar(csb[:], tsum[:], -0.5,
                                -(D / 2.0) * LN2PI,
                                op0=mybir.AluOpType.mult,
                                op1=mybir.AluOpType.add)

        # transpose A0/B0 -> zA/zB [128, DC, K]; c -> c_row [1, K]
        zA = mid_p.tile([128, DC, K], MD, tag="zA")
        zB = mid_p.tile([128, DC, K], MD, tag="zB")
        for dc in range(DC):
            tp = gm_p.tile([128, K], F32, tag="gm")
            nc.tensor.transpose(tp[:], B0[:, ts(dc, 128)], id_s[:K, :K])
            nc.vector.tensor_copy(zB[:, dc, :], tp[:])
            tp2 = gm_p.tile([128, K], F32, tag="gm")
            nc.tensor.transpose(tp2[:], A0[:, ts(dc, 128)], id_s[:K, :K])
            nc.vector.tensor_copy(zA[:, dc, :], tp2[:])
        ctp = gm_p.tile([128, K], F32, tag="gm")
        nc.tensor.transpose(ctp[:1, :], csb[:, :], id_s[:K, :K])
        crow = mid_p.tile([1, K], MD, tag="crow")
        nc.vector.tensor_copy(crow[:], ctp[:1, :])

        # ---------------- pass 2 ----------------
        for b in range(nblk):
            x2T = x2t_p.tile([128, DC, 512], MD, tag="x2T")
            nc.scalar.activation(x2T[:, 0:2, :], xTs[:, 0:2, ts(b, 512)],
                                 AF.Square)
            nc.vector.tensor_mul(x2T[:, 2:4, :], xTs[:, 2:4, ts(b, 512)],
                                 xTs[:, 2:4, ts(b, 512)])
            og = og_p.tile([128, 4, K], F32, tag="og")
            for t in range(4):
                gm = gm_p.tile([128, K], F32, tag="gm")
                for dc in range(DC):
                    nc.tensor.matmul(
                        gm[:], c(xTs[:, dc, bass.ds(b * 512 + t * 128, 128)]),
                        c(zB[:, dc, :]), start=(dc == 0), stop=False)
                for dc in range(DC):
                    nc.tensor.matmul(gm[:], c(x2T[:, dc, ts(t, 128)]),
                                     c(zA[:, dc, :]), start=False, stop=False)
                nc.tensor.matmul(gm[:], c(ones_s[:1, :128]), c(crow[:1, :]),
                                 start=False, stop=True)
                nc.scalar.activation(og[:, t, :], gm[:], AF.Copy)
            nc.sync.dma_start(
                gmm_d.rearrange("(b t p) k -> b p t k", p=128, t=4)[b],
                og[:])

    nc.compile()
    return nc


def _host_prep(x, W1, b1, W2, b2, R):
    xf = np.ascontiguousarray(x.reshape(-1, x.shape[-1]).astype(np.float32))
    Nt = xf.shape[0]
    n_cores = Nt // R
    in_maps = []
    ones = np.ones((128, 128), np.float32)
    ident = np.eye(128, dtype=np.float32)
    for ci in range(n_cores):
        sh = xf[ci * R:(ci + 1) * R]
        in_maps.append({
            "x_sh": np.ascontiguousarray(sh),
            "xT_sh": np.ascontiguousarray(sh.T),
            "w1": np.ascontiguousarray(W1.astype(np.float32)),
            "b1": np.ascontiguousarray(b1.astype(np.float32)),
            "w2": np.ascontiguousarray(W2.astype(np.float32)),
            "b2r": np.ascontiguousarray(b2.astype(np.float32)[None, :]),
            "ones": ones,
            "ident": ident,
        })
    return in_maps


_NC_CACHE = {}


def run_gmm(x, W1, b1, W2, b2, R=4096, mmdt=F32R, trace=False):
    in_maps = _host_prep(np.asarray(x), np.asarray(W1), np.asarray(b1),
                         np.asarray(W2), np.asarray(b2), R)
    n_cores = len(in_maps)
    key = (R, str(mmdt), n_cores)
    if key not in _NC_CACHE:
        _NC_CACHE[key] = build_gmm_kernel(R=R, mmdt=mmdt, n_cores=n_cores)
    nc = _NC_CACHE[key]
    try:
        res = run_bass_kernel_spmd(nc, in_maps, core_ids=list(range(n_cores)),
                                   trace=trace)
    except ModuleNotFoundError:
        # axon NTFF profile hook unavailable — run without tracing
        res = run_bass_kernel_spmd(nc, in_maps, core_ids=list(range(n_cores)),
                                   trace=False)
    gmm = np.concatenate([r["gmm"] for r in res.results], axis=0)
    var = res.results[0]["var"]
    return (gmm, var), res


def kernel(x, W1, b1, W2, b2):
    (gmm, var), _ = run_gmm(x, W1, b1, W2, b2)
    return gmm, var


if __name__ == "__main__":
    import jax
    cpu = jax.devices("cpu")[0]
    with jax.default_device(cpu):
        import reference
        inputs = reference.setup_inputs()
        inputs = {k: np.asarray(v) for k, v in inputs.items()}
    out = kernel(**inputs)
    print([o.shape for o in out])
